# revision 24
# baseline (speedup 1.0000x reference)
"""Trainium2 Bass kernel for nn_Damping (two tiny tanh-MLPs + quadratic combine).

Math (per sample, x in R^2):
    d3 = MLP_d(x)   (2 -> 32 -> 32 -> 2, tanh on hidden layers)
    o3 = MLP_o(x)   (2 -> 32 -> 32 -> 1, tanh on hidden layers)
    a = (relu(d3_0)+1e-3)*x0 ; b = (relu(d3_1)+1e-3)*x1 ; c = o3
    D0 = a*a*x0 + a*c*x1
    D1 = a*c*x0 + (c*c + b*b)*x1

Strategy: pure data-parallel over 8 cores. Per core, both branch MLPs are
merged into one 2->64->64->3 network (block-diagonal W2/W3) and two batch
sub-tiles of 512 samples are packed into the 128 partitions per matmul
(block-diagonal packed weights), so the PE and ACT engines run full width.
Activations live transposed (hidden on partitions, batch on free dim), and
matmuls run in fp16 (weights + activations; fp32 PSUM accumulate) - fp16
matmul streams 4x faster than fp32 on the PE and keeps absmax-rel error
~1e-3. All tanh runs on the ACT engine from PSUM at FD=1024 per op; ACT is
the bottleneck engine (~128 ops x ~1.04us).

The tiny L3 outputs ([6, 512] per chunk) are packed 4-chunks-per-PSUM-bank
via tile_position col-groups, bias-added + evacuated by DVE into a
per-block SBUF accumulator, then repacked through a DRAM scratch bounce
(SBUF DMA APs only allow dense partition ranges; DRAM APs allow the
strided gather) into dense [128, 512] per-quantity tiles, where the final
quadratic runs on the vector engine at full width. Outputs are written as
one interleaved [spb, 2F] tile -> a single contiguous DMA per block.

DMA count is kept to ~40/core (vs a naive ~360) because the HWDGE
descriptor ring serializes at ~0.6us per DMA and becomes the critical
path otherwise.
"""

import numpy as np

import concourse.bass as bass
import concourse.mybir as mybir
from concourse import bacc
import concourse.tile as tile
from concourse.bass_utils import run_bass_kernel_spmd

F32 = mybir.dt.float32
F16 = mybir.dt.float16
DAMP = 0.001

N_CORES = 8
B_TOTAL = 1048576
BC = B_TOTAL // N_CORES  # 131072 samples per core

F = 512  # sub-tile size = matmul free dim = one PSUM bank of fp32


def build_program(bc=BC, spb=128):
    """Build the Bass program for one core processing `bc` samples.

    spb = sub-tiles per block (the final-stage partition packing). Must
    divide bc/F and be a multiple of 8 (one PSUM-C group = 8 sub-tiles).
    """
    n_sub = bc // F
    assert bc % F == 0 and spb % 8 == 0 and n_sub % spb == 0
    n_super = bc // (4 * F)   # superchunk = 4 sub-tiles = 2 chunks
    spg = spb // 8            # groups per block
    sup_per_blk = spb // 4    # superchunks per block
    n_blocks = n_sub // spb

    nc = bacc.Bacc("TRN2", target_bir_lowering=False, debug=False)

    x = nc.dram_tensor("x", [bc, 2], F32, kind="ExternalInput")
    xt2 = nc.dram_tensor("xt2", [4, bc // 2], F16, kind="ExternalInput")
    w1p = nc.dram_tensor("w1p", [4, 128], F16, kind="ExternalInput")
    w2p = nc.dram_tensor("w2p", [128, 128], F16, kind="ExternalInput")
    w3p = nc.dram_tensor("w3p", [128, 32], F16, kind="ExternalInput")
    ball = nc.dram_tensor("ball", [128, 3], F32, kind="ExternalInput")
    y = nc.dram_tensor("y", [bc, 2], F32, kind="ExternalOutput")

    # DRAM views
    # host-packed xT: row (2t+d) holds component d of sub-tile half t,
    # chunk-major along the free dim -> big contiguous loads
    XT_CH = min(16, bc // 1024)  # chunks per xT load
    xtv = xt2[:].rearrange("r (b f) -> b r f", f=F * XT_CH)
    # per-block sample-major views (partition = sub-tile, free = (f d))
    x01v = x[:].rearrange("(b p f) d -> b p (f d)", p=spb, f=F)
    y01v = y[:].rearrange("(b p f) d -> b p (f d)", p=spb, f=F)

    Tanh = mybir.ActivationFunctionType.Tanh
    ADD = mybir.AluOpType.add
    MAX = mybir.AluOpType.max

    with tile.TileContext(nc) as tc:
        with (
            tc.tile_pool(name="wpool", bufs=1) as wpool,
            tc.tile_pool(name="xt", bufs=2) as xt_pool,
            tc.tile_pool(name="x01", bufs=2) as x01_pool,
            tc.tile_pool(name="h", bufs=2) as h_pool,
            tc.tile_pool(name="s3", bufs=2) as s3_pool,
            tc.tile_pool(name="fin", bufs=2) as fin_pool,
            tc.tile_pool(name="tmp", bufs=1) as tmp_pool,
            tc.tile_pool(name="dout", bufs=2) as out_pool,
            tc.tile_pool(name="psA", bufs=2, space=bass.MemorySpace.PSUM) as psumA,
            tc.tile_pool(name="psB", bufs=1, space=bass.MemorySpace.PSUM) as psumB,
            tc.tile_pool(name="psC", bufs=2, space=bass.MemorySpace.PSUM) as psumC,
            tc.tile_pool(name="scr", bufs=3, space=bass.MemorySpace.DRAM) as scr_pool,
        ):
            w1s = wpool.tile([4, 128], F16, tag="w1s")
            w2s = wpool.tile([128, 128], F16, tag="w2s")
            w3s = wpool.tile([128, 32], F16, tag="w3s")
            balls = wpool.tile([128, 3], F32, tag="balls")
            nc.sync.dma_start(w1s[:], w1p[:])
            nc.sync.dma_start(w2s[:], w2p[:])
            nc.sync.dma_start(w3s[:], w3p[:])
            nc.sync.dma_start(balls[:], ball[:])
            b1s = balls[:, 0:1]
            b2s = balls[:, 1:2]
            bc3s = balls[:, 2:3]

            # warm the ACT tanh table (~2.7us load) concurrently with the
            # initial input DMAs instead of stalling the first real tanh
            warm = wpool.tile([1, 16], F32, tag="warm")
            nc.gpsimd.memset(warm[:], 0.0)
            nc.scalar.activation(warm[:], warm[:], Tanh)

            psC = None
            blk_tiles = None

            for g in range(n_super):
                blk = g // sup_per_blk
                gq = g % sup_per_blk

                if gq == 0:
                    x01t = x01_pool.tile([spb, 2 * F], F32, tag="x01")
                    d30a = fin_pool.tile([spb, F], F32, tag="d30")
                    d31a = fin_pool.tile([spb, F], F32, tag="d31")
                    o3a = fin_pool.tile([spb, F], F32, tag="o3")
                    s3big = s3_pool.tile([128, F * spg], F32, tag="s3big")
                    scrb = scr_pool.tile([3, spb, F], F32, tag="scrb")
                    blk_tiles = (x01t, d30a, d31a, o3a, s3big, scrb)
                x01t, d30a, d31a, o3a, s3big, scrb = blk_tiles
                if gq == min(1, sup_per_blk - 1):
                    # x01 is only consumed by the block-final stage; load it
                    # off the startup critical path
                    nc.sync.dma_start(x01t[:], x01v[blk])

                # ---- load xT for 16 chunks at a time (8 superchunks)
                if g % (XT_CH // 2) == 0:
                    xtb = xt_pool.tile([4, F * XT_CH], F16, tag="xt")
                    nc.sync.dma_start(xtb[:], xtv[g // (XT_CH // 2)])
                xts = [
                    xtb[:, ((2 * g + j) % XT_CH) * F : ((2 * g + j) % XT_CH + 1) * F]
                    for j in range(2)
                ]

                # ---- L1 (fp16): [4,128]^T @ [4,F] -> [128,F]
                h1 = h_pool.tile([128, 2 * F], F16, tag="h1")
                if act1_split:
                    for j in range(2):
                        psA = psumA.tile([128, F], F32, tag="psA")
                        nc.tensor.matmul(
                            psA[:], w1s[:], xts[j], start=True, stop=True,
                        )
                        nc.scalar.activation(
                            h1[:, j * F : (j + 1) * F], psA[:], Tanh, bias=b1s
                        )
                else:
                    psA = psumA.tile([128, 2 * F], F32, tag="psA")
                    for j in range(2):
                        nc.tensor.matmul(
                            psA[:, j * F : (j + 1) * F], w1s[:], xts[j],
                            start=True, stop=True,
                        )
                    nc.scalar.activation(h1[:], psA[:], Tanh, bias=b1s)

                # ---- L2: [128,128]^T @ [128,F] -> [128,F]
                psB = psumB.tile([128, 2 * F], F32, tag="psB")
                for j in range(2):
                    nc.tensor.matmul(
                        psB[:, j * F : (j + 1) * F], w2s[:], h1[:, j * F : (j + 1) * F],
                        start=True, stop=True,
                    )
                h2 = h_pool.tile([128, 2 * F], F16, tag="h2")
                nc.scalar.activation(h2[:], psB[:], Tanh, bias=b2s)

                # ---- L3: [128,6]^T @ [128,F] -> [6,F] at col-group jj
                if g % 2 == 0:
                    psC = psumC.tile([128, F], F32, tag="psC")
                for j in range(2):
                    jj = 2 * (g % 2) + j
                    nc.tensor.matmul(
                        psC[32 * jj : 32 * jj + 32, :], w3s[:],
                        h2[:, j * F : (j + 1) * F],
                        start=True, stop=True, tile_position=(0, 32 * jj),
                    )

                # ---- evacuate psC (4 chunks) into the block s3 accumulator
                if g % 2 == 1:
                    q2l = (g // 2) % spg  # group index within block
                    nc.vector.tensor_scalar(
                        s3big[:, q2l * F : (q2l + 1) * F], psC[:],
                        bc3s, None, ADD,
                    )

                # ---- block-level repack via DRAM scratch:
                # scratch row order = (q, j, k) = destination partition order
                if gq == sup_per_blk - 1:
                    scrv = scrb[:].rearrange("m (q r) f -> m q r f", r=8)
                    for j in range(4):
                        for k in range(2):
                            nc.sync.dma_start(
                                scrv[:, :, 2 * j + k, :],
                                s3big[32 * j + 3 * k : 32 * j + 3 * k + 3, :],
                            )
                    for m, dst_t in enumerate((d30a, d31a, o3a)):
                        nc.sync.dma_start(dst_t[:], scrb[m])

                # ---- final quadratic stage at end of block
                if gq == sup_per_blk - 1:
                    xv = x01t[:].rearrange("p (f d) -> p f d", d=2)
                    x0, x1 = xv[:, :, 0], xv[:, :, 1]

                    def T(tag):
                        return tmp_pool.tile([spb, F], F32, tag=tag, name=tag)

                    r0 = T("r0")
                    nc.vector.tensor_scalar(r0[:], d30a[:], 0.0, DAMP, MAX, ADD)
                    r1 = T("r1")
                    nc.vector.tensor_scalar(r1[:], d31a[:], 0.0, DAMP, MAX, ADD)
                    a_ = T("a")
                    nc.vector.tensor_mul(a_[:], r0[:], x0)
                    b_ = T("b")
                    nc.vector.tensor_mul(b_[:], r1[:], x1)
                    t1 = T("t1")
                    nc.vector.tensor_mul(t1[:], a_[:], x0)
                    t2 = T("t2")
                    nc.vector.tensor_mul(t2[:], o3a[:], x1)
                    s_ = T("s")
                    nc.vector.tensor_add(s_[:], t1[:], t2[:])

                    D01 = out_pool.tile([spb, 2 * F], F32, tag="D01")
                    dv = D01[:].rearrange("p (f d) -> p f d", d=2)
                    D0v, D1v = dv[:, :, 0], dv[:, :, 1]
                    nc.vector.tensor_mul(D0v, a_[:], s_[:])

                    w_ = T("w")
                    nc.vector.tensor_mul(w_[:], o3a[:], t1[:])
                    cc = T("cc")
                    nc.vector.tensor_mul(cc[:], o3a[:], o3a[:])
                    bb = T("bb")
                    nc.vector.tensor_mul(bb[:], b_[:], b_[:])
                    u_ = T("u")
                    nc.vector.tensor_add(u_[:], cc[:], bb[:])
                    v_ = T("v")
                    nc.vector.tensor_mul(v_[:], u_[:], x1)
                    nc.vector.tensor_add(D1v, w_[:], v_[:])

                    nc.sync.dma_start(y01v[blk], D01[:])

    nc.compile()
    return nc


def pack_weights(inputs):
    """Host-side packing of the tiny MLP weights into block-diag layout."""
    g = lambda k: np.asarray(inputs[k], dtype=np.float32)
    w_d1, w_d2, w_d3 = g("w_d1"), g("w_d2"), g("w_d3")
    w_o1, w_o2, w_o3 = g("w_o1"), g("w_o2"), g("w_o3")
    b_d1, b_d2, b_d3 = g("b_d1"), g("b_d2"), g("b_d3")
    b_o1, b_o2, b_o3 = g("b_o1"), g("b_o2"), g("b_o3")

    W1 = np.concatenate([w_d1, w_o1], axis=1)  # [2, 64]
    W2 = np.zeros((64, 64), np.float32)
    W2[:32, :32] = w_d2
    W2[32:, 32:] = w_o2
    W3 = np.zeros((64, 3), np.float32)
    W3[:32, 0:2] = w_d3
    W3[32:, 2:3] = w_o3

    W1p = np.zeros((4, 128), np.float32)
    W1p[0:2, 0:64] = W1
    W1p[2:4, 64:128] = W1
    W2p = np.zeros((128, 128), np.float32)
    W2p[:64, :64] = W2
    W2p[64:, 64:] = W2
    W3p = np.zeros((128, 32), np.float32)
    W3p[:64, 0:3] = W3
    W3p[64:, 3:6] = W3

    B1 = np.concatenate([b_d1, b_o1])  # [64]
    B1p = np.tile(B1, 2)[:, None].astype(np.float32)
    B2 = np.concatenate([b_d2, b_o2])
    B2p = np.tile(B2, 2)[:, None].astype(np.float32)

    bc3 = np.zeros((128, 1), np.float32)
    vals = [b_d3[0], b_d3[1], b_o3[0]]
    for r in range(128):
        if r % 32 < 6:
            bc3[r, 0] = vals[(r % 32) % 3]

    ball = np.concatenate([B1p, B2p, bc3], axis=1).astype(np.float32)
    return {
        "w1p": W1p.astype(np.float16),
        "w2p": W2p.astype(np.float16),
        "w3p": W3p.astype(np.float16),
        "ball": np.ascontiguousarray(ball),
    }


_CACHE = {}


def _get_program(bc, spb):
    key = (bc, spb)
    if key not in _CACHE:
        _CACHE[key] = build_program(bc, spb)
    return _CACHE[key]


LAST_RESULTS = None


def run(inputs, trace=False, n_cores=N_CORES):
    global LAST_RESULTS
    x = np.ascontiguousarray(np.asarray(inputs["x"], dtype=np.float32))
    B = x.shape[0]
    bc = B // n_cores
    packed = pack_weights(inputs)
    nc = _get_program(bc, 128 if bc % (128 * F) == 0 else 8)

    in_maps = []
    for i in range(n_cores):
        xs = np.ascontiguousarray(x[i * bc : (i + 1) * bc])
        # XTP[2t+d, c*F+f] = xs[c*2F + t*F + f, d]
        v = xs.reshape(bc // 1024, 2, 512, 2)
        xtp = np.ascontiguousarray(
            v.transpose(1, 3, 0, 2).reshape(4, bc // 2).astype(np.float16)
        )
        m = {"x": xs, "xt2": xtp}
        m.update(packed)
        in_maps.append(m)

    res = run_bass_kernel_spmd(
        nc, in_maps, core_ids=list(range(n_cores)), trace=trace
    )
    LAST_RESULTS = res
    y = np.concatenate([res.results[i]["y"] for i in range(n_cores)], axis=0)
    return y


def kernel(**inputs) -> np.ndarray:
    return run(inputs, trace=False)


# revision 31
# speedup vs baseline: 1.0078x; 1.0078x over previous
"""Trainium2 Bass kernel for nn_Damping (two tiny tanh-MLPs + quadratic combine).

Math (per sample, x in R^2):
    d3 = MLP_d(x)   (2 -> 32 -> 32 -> 2, tanh on hidden layers)
    o3 = MLP_o(x)   (2 -> 32 -> 32 -> 1, tanh on hidden layers)
    a = (relu(d3_0)+1e-3)*x0 ; b = (relu(d3_1)+1e-3)*x1 ; c = o3
    D0 = a*a*x0 + a*c*x1
    D1 = a*c*x0 + (c*c + b*b)*x1

Strategy: pure data-parallel over 8 cores. Per core, both branch MLPs are
merged into one 2->64->64->3 network (block-diagonal W2/W3) and two batch
sub-tiles of 512 samples are packed into the 128 partitions per matmul
(block-diagonal packed weights), so the PE and ACT engines run full width.
Activations live transposed (hidden on partitions, batch on free dim), and
matmuls run in fp16 (weights + activations; fp32 PSUM accumulate) - fp16
matmul streams 4x faster than fp32 on the PE and keeps absmax-rel error
~1e-3. All tanh runs on the ACT engine from PSUM at FD=1024 per op; ACT is
the bottleneck engine (~128 ops x ~1.04us).

The tiny L3 outputs ([6, 512] per chunk) are packed 4-chunks-per-PSUM-bank
via tile_position col-groups, bias-added + evacuated by DVE into a
per-block SBUF accumulator, then repacked through a DRAM scratch bounce
(SBUF DMA APs only allow dense partition ranges; DRAM APs allow the
strided gather) into dense [128, 512] per-quantity tiles, where the final
quadratic runs on the vector engine at full width. Outputs are written as
one interleaved [spb, 2F] tile -> a single contiguous DMA per block.

DMA count is kept to ~40/core (vs a naive ~360) because the HWDGE
descriptor ring serializes at ~0.6us per DMA and becomes the critical
path otherwise.
"""

import numpy as np

import concourse.bass as bass
import concourse.mybir as mybir
from concourse import bacc
import concourse.tile as tile
from concourse.bass_utils import run_bass_kernel_spmd

F32 = mybir.dt.float32
F16 = mybir.dt.float16
DAMP = 0.001

N_CORES = 8
B_TOTAL = 1048576
BC = B_TOTAL // N_CORES  # 131072 samples per core

F = 512  # sub-tile size = matmul free dim = one PSUM bank of fp32


def build_program(bc=BC, spb=128):
    """Build the Bass program for one core processing `bc` samples.

    spb = sub-tiles per block (the final-stage partition packing). Must
    divide bc/F and be a multiple of 8 (one PSUM-C group = 8 sub-tiles).
    """
    n_sub = bc // F
    assert bc % F == 0 and spb % 8 == 0 and n_sub % spb == 0
    n_super = bc // (4 * F)   # superchunk = 4 sub-tiles = 2 chunks
    spg = spb // 8            # groups per block
    sup_per_blk = spb // 4    # superchunks per block
    n_blocks = n_sub // spb

    nc = bacc.Bacc("TRN2", target_bir_lowering=False, debug=False)

    x = nc.dram_tensor("x", [bc, 2], F32, kind="ExternalInput")
    xt2 = nc.dram_tensor("xt2", [4, bc // 2], F16, kind="ExternalInput")
    w1p = nc.dram_tensor("w1p", [4, 128], F16, kind="ExternalInput")
    w2p = nc.dram_tensor("w2p", [128, 128], F16, kind="ExternalInput")
    w3p = nc.dram_tensor("w3p", [128, 32], F16, kind="ExternalInput")
    ball = nc.dram_tensor("ball", [128, 3], F32, kind="ExternalInput")
    y = nc.dram_tensor("y", [bc, 2], F32, kind="ExternalOutput")

    # DRAM views
    # host-packed xT: row (2t+d) holds component d of sub-tile half t,
    # chunk-major along the free dim -> big contiguous loads
    XT_CH = min(16, bc // 1024)  # chunks per xT load
    xtv = xt2[:].rearrange("r (b f) -> b r f", f=F * XT_CH)
    # per-block sample-major views (partition = sub-tile, free = (f d))
    x01v = x[:].rearrange("(b p f) d -> b p (f d)", p=spb, f=F)
    y01v = y[:].rearrange("(b p f) d -> b p (f d)", p=spb, f=F)

    Tanh = mybir.ActivationFunctionType.Tanh
    ADD = mybir.AluOpType.add
    MAX = mybir.AluOpType.max

    with tile.TileContext(nc) as tc:
        with (
            tc.tile_pool(name="wpool", bufs=1) as wpool,
            tc.tile_pool(name="xt", bufs=2) as xt_pool,
            tc.tile_pool(name="x01", bufs=2) as x01_pool,
            tc.tile_pool(name="h", bufs=2) as h_pool,
            tc.tile_pool(name="s3", bufs=2) as s3_pool,
            tc.tile_pool(name="fin", bufs=2) as fin_pool,
            tc.tile_pool(name="tmp", bufs=1) as tmp_pool,
            tc.tile_pool(name="dout", bufs=2) as out_pool,
            tc.tile_pool(name="psA", bufs=2, space=bass.MemorySpace.PSUM) as psumA,
            tc.tile_pool(name="psB", bufs=1, space=bass.MemorySpace.PSUM) as psumB,
            tc.tile_pool(name="psC", bufs=2, space=bass.MemorySpace.PSUM) as psumC,
            tc.tile_pool(name="scr", bufs=3, space=bass.MemorySpace.DRAM) as scr_pool,
        ):
            w1s = wpool.tile([4, 128], F16, tag="w1s")
            w2s = wpool.tile([128, 128], F16, tag="w2s")
            w3s = wpool.tile([128, 32], F16, tag="w3s")
            balls = wpool.tile([128, 3], F32, tag="balls")
            nc.sync.dma_start(w1s[:], w1p[:])
            nc.sync.dma_start(w2s[:], w2p[:])
            nc.sync.dma_start(w3s[:], w3p[:])
            nc.sync.dma_start(balls[:], ball[:])
            b1s = balls[:, 0:1]
            b2s = balls[:, 1:2]
            bc3s = balls[:, 2:3]

            # warm the ACT tanh table (~2.7us load) concurrently with the
            # initial input DMAs instead of stalling the first real tanh
            warm = wpool.tile([1, 16], F32, tag="warm")
            nc.gpsimd.memset(warm[:], 0.0)
            nc.scalar.activation(warm[:], warm[:], Tanh)

            psC = None
            blk_tiles = None

            for g in range(n_super):
                blk = g // sup_per_blk
                gq = g % sup_per_blk

                if gq == 0:
                    x01t = x01_pool.tile([spb, 2 * F], F32, tag="x01")
                    d30a = fin_pool.tile([spb, F], F32, tag="d30")
                    d31a = fin_pool.tile([spb, F], F32, tag="d31")
                    o3a = fin_pool.tile([spb, F], F32, tag="o3")
                    s3big = s3_pool.tile([128, F * spg], F32, tag="s3big")
                    scrb = scr_pool.tile([3, spb, F], F32, tag="scrb")
                    blk_tiles = (x01t, d30a, d31a, o3a, s3big, scrb)
                x01t, d30a, d31a, o3a, s3big, scrb = blk_tiles
                if gq == min(1, sup_per_blk - 1):
                    # x01 is only consumed by the block-final stage; load it
                    # off the startup critical path
                    nc.sync.dma_start(x01t[:], x01v[blk])

                # ---- load xT for 16 chunks at a time (8 superchunks)
                if g % (XT_CH // 2) == 0:
                    xtb = xt_pool.tile([4, F * XT_CH], F16, tag="xt")
                    nc.sync.dma_start(xtb[:], xtv[g // (XT_CH // 2)])
                xts = [
                    xtb[:, ((2 * g + j) % XT_CH) * F : ((2 * g + j) % XT_CH + 1) * F]
                    for j in range(2)
                ]

                # ---- L1 (fp16): [4,128]^T @ [4,F] -> [128,F]
                h1 = h_pool.tile([128, 2 * F], F16, tag="h1")
                if act1_split:
                    for j in range(2):
                        psA = psumA.tile([128, F], F32, tag="psA")
                        nc.tensor.matmul(
                            psA[:], w1s[:], xts[j], start=True, stop=True,
                        )
                        nc.scalar.activation(
                            h1[:, j * F : (j + 1) * F], psA[:], Tanh, bias=b1s
                        )
                else:
                    psA = psumA.tile([128, 2 * F], F32, tag="psA")
                    for j in range(2):
                        nc.tensor.matmul(
                            psA[:, j * F : (j + 1) * F], w1s[:], xts[j],
                            start=True, stop=True,
                        )
                    nc.scalar.activation(h1[:], psA[:], Tanh, bias=b1s)

                # ---- L2: [128,128]^T @ [128,F] -> [128,F]
                psB = psumB.tile([128, 2 * F], F32, tag="psB")
                for j in range(2):
                    nc.tensor.matmul(
                        psB[:, j * F : (j + 1) * F], w2s[:], h1[:, j * F : (j + 1) * F],
                        start=True, stop=True,
                    )
                h2 = h_pool.tile([128, 2 * F], F16, tag="h2")
                nc.scalar.activation(h2[:], psB[:], Tanh, bias=b2s)

                # ---- L3: [128,6]^T @ [128,F] -> [6,F] at col-group jj
                if g % 2 == 0:
                    psC = psumC.tile([128, F], F32, tag="psC")
                for j in range(2):
                    jj = 2 * (g % 2) + j
                    nc.tensor.matmul(
                        psC[32 * jj : 32 * jj + 32, :], w3s[:],
                        h2[:, j * F : (j + 1) * F],
                        start=True, stop=True, tile_position=(0, 32 * jj),
                    )

                # ---- evacuate psC (4 chunks) into the block s3 accumulator
                if g % 2 == 1:
                    q2l = (g // 2) % spg  # group index within block
                    nc.vector.tensor_scalar(
                        s3big[:, q2l * F : (q2l + 1) * F], psC[:],
                        bc3s, None, ADD,
                    )

                # ---- block-level repack via DRAM scratch:
                # scratch row order = (q, j, k) = destination partition order
                if gq == sup_per_blk - 1:
                    scrv = scrb[:].rearrange("m (q r) f -> m q r f", r=8)
                    for j in range(4):
                        for k in range(2):
                            nc.sync.dma_start(
                                scrv[:, :, 2 * j + k, :],
                                s3big[32 * j + 3 * k : 32 * j + 3 * k + 3, :],
                            )
                    for m, dst_t in enumerate((d30a, d31a, o3a)):
                        nc.sync.dma_start(dst_t[:], scrb[m])

                # ---- final quadratic stage at end of block
                if gq == sup_per_blk - 1:
                    xv = x01t[:].rearrange("p (f d) -> p f d", d=2)
                    x0, x1 = xv[:, :, 0], xv[:, :, 1]

                    def T(tag):
                        return tmp_pool.tile([spb, F], F32, tag=tag, name=tag)

                    r0 = T("r0")
                    nc.vector.tensor_scalar(r0[:], d30a[:], 0.0, DAMP, MAX, ADD)
                    r1 = T("r1")
                    nc.vector.tensor_scalar(r1[:], d31a[:], 0.0, DAMP, MAX, ADD)
                    a_ = T("a")
                    nc.vector.tensor_mul(a_[:], r0[:], x0)
                    b_ = T("b")
                    nc.vector.tensor_mul(b_[:], r1[:], x1)
                    t1 = T("t1")
                    nc.vector.tensor_mul(t1[:], a_[:], x0)
                    t2 = T("t2")
                    nc.vector.tensor_mul(t2[:], o3a[:], x1)
                    s_ = T("s")
                    nc.vector.tensor_add(s_[:], t1[:], t2[:])

                    D01 = out_pool.tile([spb, 2 * F], F32, tag="D01")
                    dv = D01[:].rearrange("p (f d) -> p f d", d=2)
                    D0v, D1v = dv[:, :, 0], dv[:, :, 1]
                    nc.vector.tensor_mul(D0v, a_[:], s_[:])

                    # D1 = c*s + b*(b*x1)  (c*s = a*c*x0 + c^2*x1)
                    bx1 = T("bx1")
                    nc.vector.tensor_mul(bx1[:], b_[:], x1)
                    m1 = T("m1")
                    nc.vector.tensor_mul(m1[:], o3a[:], s_[:])
                    m2 = T("m2")
                    nc.vector.tensor_mul(m2[:], b_[:], bx1[:])
                    nc.vector.tensor_add(D1v, m1[:], m2[:])

                    nc.sync.dma_start(y01v[blk], D01[:])

    nc.compile()
    return nc


def pack_weights(inputs):
    """Host-side packing of the tiny MLP weights into block-diag layout."""
    g = lambda k: np.asarray(inputs[k], dtype=np.float32)
    w_d1, w_d2, w_d3 = g("w_d1"), g("w_d2"), g("w_d3")
    w_o1, w_o2, w_o3 = g("w_o1"), g("w_o2"), g("w_o3")
    b_d1, b_d2, b_d3 = g("b_d1"), g("b_d2"), g("b_d3")
    b_o1, b_o2, b_o3 = g("b_o1"), g("b_o2"), g("b_o3")

    W1 = np.concatenate([w_d1, w_o1], axis=1)  # [2, 64]
    W2 = np.zeros((64, 64), np.float32)
    W2[:32, :32] = w_d2
    W2[32:, 32:] = w_o2
    W3 = np.zeros((64, 3), np.float32)
    W3[:32, 0:2] = w_d3
    W3[32:, 2:3] = w_o3

    W1p = np.zeros((4, 128), np.float32)
    W1p[0:2, 0:64] = W1
    W1p[2:4, 64:128] = W1
    W2p = np.zeros((128, 128), np.float32)
    W2p[:64, :64] = W2
    W2p[64:, 64:] = W2
    W3p = np.zeros((128, 32), np.float32)
    W3p[:64, 0:3] = W3
    W3p[64:, 3:6] = W3

    B1 = np.concatenate([b_d1, b_o1])  # [64]
    B1p = np.tile(B1, 2)[:, None].astype(np.float32)
    B2 = np.concatenate([b_d2, b_o2])
    B2p = np.tile(B2, 2)[:, None].astype(np.float32)

    bc3 = np.zeros((128, 1), np.float32)
    vals = [b_d3[0], b_d3[1], b_o3[0]]
    for r in range(128):
        if r % 32 < 6:
            bc3[r, 0] = vals[(r % 32) % 3]

    ball = np.concatenate([B1p, B2p, bc3], axis=1).astype(np.float32)
    return {
        "w1p": W1p.astype(np.float16),
        "w2p": W2p.astype(np.float16),
        "w3p": W3p.astype(np.float16),
        "ball": np.ascontiguousarray(ball),
    }


_CACHE = {}


def _get_program(bc, spb):
    key = (bc, spb)
    if key not in _CACHE:
        _CACHE[key] = build_program(bc, spb)
    return _CACHE[key]


LAST_RESULTS = None


def run(inputs, trace=False, n_cores=N_CORES):
    global LAST_RESULTS
    x = np.ascontiguousarray(np.asarray(inputs["x"], dtype=np.float32))
    B = x.shape[0]
    bc = B // n_cores
    packed = pack_weights(inputs)
    nc = _get_program(bc, 128 if bc % (128 * F) == 0 else 8)

    in_maps = []
    for i in range(n_cores):
        xs = np.ascontiguousarray(x[i * bc : (i + 1) * bc])
        # XTP[2t+d, c*F+f] = xs[c*2F + t*F + f, d]
        v = xs.reshape(bc // 1024, 2, 512, 2)
        xtp = np.ascontiguousarray(
            v.transpose(1, 3, 0, 2).reshape(4, bc // 2).astype(np.float16)
        )
        m = {"x": xs, "xt2": xtp}
        m.update(packed)
        in_maps.append(m)

    res = run_bass_kernel_spmd(
        nc, in_maps, core_ids=list(range(n_cores)), trace=trace
    )
    LAST_RESULTS = res
    y = np.concatenate([res.results[i]["y"] for i in range(n_cores)], axis=0)
    return y


def kernel(**inputs) -> np.ndarray:
    return run(inputs, trace=False)


# revision 34
# speedup vs baseline: 1.0140x; 1.0062x over previous
"""Trainium2 Bass kernel for nn_Damping (two tiny tanh-MLPs + quadratic combine).

Math (per sample, x in R^2):
    d3 = MLP_d(x)   (2 -> 32 -> 32 -> 2, tanh on hidden layers)
    o3 = MLP_o(x)   (2 -> 32 -> 32 -> 1, tanh on hidden layers)
    a = (relu(d3_0)+1e-3)*x0 ; b = (relu(d3_1)+1e-3)*x1 ; c = o3
    D0 = a*a*x0 + a*c*x1
    D1 = a*c*x0 + (c*c + b*b)*x1

Strategy: pure data-parallel over 8 cores. Per core, both branch MLPs are
merged into one 2->64->64->3 network (block-diagonal W2/W3) and two batch
sub-tiles of 512 samples are packed into the 128 partitions per matmul
(block-diagonal packed weights), so the PE and ACT engines run full width.
Activations live transposed (hidden on partitions, batch on free dim), and
matmuls run in fp16 (weights + activations; fp32 PSUM accumulate) - fp16
matmul streams 4x faster than fp32 on the PE and keeps absmax-rel error
~1e-3. All tanh runs on the ACT engine from PSUM at FD=1024 per op; ACT is
the bottleneck engine (~128 ops x ~1.04us).

The tiny L3 outputs ([6, 512] per chunk) are packed 4-chunks-per-PSUM-bank
via tile_position col-groups, bias-added + evacuated by DVE into a
per-block SBUF accumulator, then repacked through a DRAM scratch bounce
(SBUF DMA APs only allow dense partition ranges; DRAM APs allow the
strided gather) into dense [128, 512] per-quantity tiles, where the final
quadratic runs on the vector engine at full width. Outputs are written as
one interleaved [spb, 2F] tile -> a single contiguous DMA per block.

DMA count is kept to ~40/core (vs a naive ~360) because the HWDGE
descriptor ring serializes at ~0.6us per DMA and becomes the critical
path otherwise.
"""

import numpy as np

import concourse.bass as bass
import concourse.mybir as mybir
from concourse import bacc
import concourse.tile as tile
from concourse.bass_utils import run_bass_kernel_spmd

F32 = mybir.dt.float32
F16 = mybir.dt.float16
DAMP = 0.001

N_CORES = 8
B_TOTAL = 1048576
BC = B_TOTAL // N_CORES  # 131072 samples per core

F = 512  # sub-tile size = matmul free dim = one PSUM bank of fp32


def build_program(bc=BC, spb=128):
    """Build the Bass program for one core processing `bc` samples.

    spb = sub-tiles per block (the final-stage partition packing). Must
    divide bc/F and be a multiple of 8 (one PSUM-C group = 8 sub-tiles).
    """
    n_sub = bc // F
    assert bc % F == 0 and spb % 8 == 0 and n_sub % spb == 0
    n_super = bc // (4 * F)   # superchunk = 4 sub-tiles = 2 chunks
    spg = spb // 8            # groups per block
    sup_per_blk = spb // 4    # superchunks per block
    n_blocks = n_sub // spb

    nc = bacc.Bacc("TRN2", target_bir_lowering=False, debug=False)

    x = nc.dram_tensor("x", [bc, 2], F32, kind="ExternalInput")
    xt2 = nc.dram_tensor("xt2", [4, bc // 2], F16, kind="ExternalInput")
    w1p = nc.dram_tensor("w1p", [4, 128], F16, kind="ExternalInput")
    w2p = nc.dram_tensor("w2p", [128, 128], F16, kind="ExternalInput")
    w3p = nc.dram_tensor("w3p", [128, 32], F16, kind="ExternalInput")
    ball = nc.dram_tensor("ball", [128, 3], F32, kind="ExternalInput")
    y = nc.dram_tensor("y", [bc, 2], F32, kind="ExternalOutput")

    # DRAM views
    # host-packed xT: row (2t+d) holds component d of sub-tile half t,
    # chunk-major along the free dim -> big contiguous loads
    XT_CH = min(16, bc // 1024)  # chunks per xT load
    xtv = xt2[:].rearrange("r (b f) -> b r f", f=F * XT_CH)
    # per-block sample-major views (partition = sub-tile, free = (f d))
    x01v = x[:].rearrange("(b p f) d -> b p (f d)", p=spb, f=F)
    y01v = y[:].rearrange("(b p f) d -> b p (f d)", p=spb, f=F)

    Tanh = mybir.ActivationFunctionType.Tanh
    ADD = mybir.AluOpType.add
    MAX = mybir.AluOpType.max

    with tile.TileContext(nc) as tc:
        with (
            tc.tile_pool(name="wpool", bufs=1) as wpool,
            tc.tile_pool(name="xt", bufs=2) as xt_pool,
            tc.tile_pool(name="x01", bufs=2) as x01_pool,
            tc.tile_pool(name="h", bufs=2) as h_pool,
            tc.tile_pool(name="s3", bufs=2) as s3_pool,
            tc.tile_pool(name="fin", bufs=2) as fin_pool,
            tc.tile_pool(name="tmp", bufs=1) as tmp_pool,
            tc.tile_pool(name="dout", bufs=2) as out_pool,
            tc.tile_pool(name="psA", bufs=2, space=bass.MemorySpace.PSUM) as psumA,
            tc.tile_pool(name="psB", bufs=1, space=bass.MemorySpace.PSUM) as psumB,
            tc.tile_pool(name="psC", bufs=2, space=bass.MemorySpace.PSUM) as psumC,
            tc.tile_pool(name="scr", bufs=3, space=bass.MemorySpace.DRAM) as scr_pool,
        ):
            w1s = wpool.tile([4, 128], F16, tag="w1s")
            w2s = wpool.tile([128, 128], F16, tag="w2s")
            w3s = wpool.tile([128, 32], F16, tag="w3s")
            balls = wpool.tile([128, 3], F32, tag="balls")
            nc.sync.dma_start(w1s[:], w1p[:])
            nc.sync.dma_start(w2s[:], w2p[:])
            nc.sync.dma_start(w3s[:], w3p[:])
            nc.sync.dma_start(balls[:], ball[:])
            b1s = balls[:, 0:1]
            b2s = balls[:, 1:2]
            bc3s = balls[:, 2:3]

            # warm the ACT tanh table (~2.7us load) concurrently with the
            # initial input DMAs instead of stalling the first real tanh
            warm = wpool.tile([1, 16], F32, tag="warm")
            nc.gpsimd.memset(warm[:], 0.0)
            nc.scalar.activation(warm[:], warm[:], Tanh)

            psC = None
            blk_tiles = None

            for g in range(n_super):
                blk = g // sup_per_blk
                gq = g % sup_per_blk

                if gq == 0:
                    x01t = x01_pool.tile([spb, 2 * F], F32, tag="x01")
                    d30a = fin_pool.tile([spb, F], F32, tag="d30")
                    d31a = fin_pool.tile([spb, F], F32, tag="d31")
                    o3a = fin_pool.tile([spb, F], F32, tag="o3")
                    s3big = s3_pool.tile([128, F * spg], F32, tag="s3big")
                    scrb = scr_pool.tile([3, spb, F], F32, tag="scrb")
                    blk_tiles = (x01t, d30a, d31a, o3a, s3big, scrb)
                x01t, d30a, d31a, o3a, s3big, scrb = blk_tiles
                if gq == min(1, sup_per_blk - 1):
                    # x01 is only consumed by the block-final stage; load it
                    # off the startup critical path
                    nc.sync.dma_start(x01t[:], x01v[blk])

                # ---- load xT for 16 chunks at a time (8 superchunks)
                if g % (XT_CH // 2) == 0:
                    xtb = xt_pool.tile([4, F * XT_CH], F16, tag="xt")
                    nc.sync.dma_start(xtb[:], xtv[g // (XT_CH // 2)])
                xts = [
                    xtb[:, ((2 * g + j) % XT_CH) * F : ((2 * g + j) % XT_CH + 1) * F]
                    for j in range(2)
                ]

                # ---- L1 (fp16): [4,128]^T @ [4,F] -> [128,F]
                h1 = h_pool.tile([128, 2 * F], F16, tag="h1")
                if act1_split:
                    for j in range(2):
                        psA = psumA.tile([128, F], F32, tag="psA")
                        nc.tensor.matmul(
                            psA[:], w1s[:], xts[j], start=True, stop=True,
                        )
                        nc.scalar.activation(
                            h1[:, j * F : (j + 1) * F], psA[:], Tanh, bias=b1s
                        )
                else:
                    psA = psumA.tile([128, 2 * F], F32, tag="psA")
                    for j in range(2):
                        nc.tensor.matmul(
                            psA[:, j * F : (j + 1) * F], w1s[:], xts[j],
                            start=True, stop=True,
                        )
                    nc.scalar.activation(h1[:], psA[:], Tanh, bias=b1s)

                # ---- L2: [128,128]^T @ [128,F] -> [128,F]
                psB = psumB.tile([128, 2 * F], F32, tag="psB")
                for j in range(2):
                    nc.tensor.matmul(
                        psB[:, j * F : (j + 1) * F], w2s[:], h1[:, j * F : (j + 1) * F],
                        start=True, stop=True,
                    )
                h2 = h_pool.tile([128, 2 * F], F16, tag="h2")
                nc.scalar.activation(h2[:], psB[:], Tanh, bias=b2s)

                # ---- L3: [128,6]^T @ [128,F] -> [6,F] at col-group jj
                if g % 2 == 0:
                    psC = psumC.tile([128, F], F32, tag="psC")
                for j in range(2):
                    jj = 2 * (g % 2) + j
                    nc.tensor.matmul(
                        psC[32 * jj : 32 * jj + 32, :], w3s[:],
                        h2[:, j * F : (j + 1) * F],
                        start=True, stop=True, tile_position=(0, 32 * jj),
                    )

                # ---- evacuate psC (4 chunks) into the block s3 accumulator
                if g % 2 == 1:
                    q2l = (g // 2) % spg  # group index within block
                    nc.vector.tensor_scalar(
                        s3big[:, q2l * F : (q2l + 1) * F], psC[:],
                        bc3s, None, ADD,
                    )

                # ---- block-level repack via DRAM scratch:
                # scratch row order = (q, j, k) = destination partition order
                if gq == sup_per_blk - 1:
                    scrv = scrb[:].rearrange("m (q r) f -> m q r f", r=8)
                    for j in range(4):
                        for k in range(2):
                            nc.sync.dma_start(
                                scrv[:, :, 2 * j + k, :],
                                s3big[32 * j + 3 * k : 32 * j + 3 * k + 3, :],
                            )
                    for m, dst_t in enumerate((d30a, d31a, o3a)):
                        nc.sync.dma_start(dst_t[:], scrb[m])

                # ---- final quadratic stage at end of block
                if gq == sup_per_blk - 1:
                    xv = x01t[:].rearrange("p (f d) -> p f d", d=2)
                    x0, x1 = xv[:, :, 0], xv[:, :, 1]

                    def T(tag):
                        return tmp_pool.tile([spb, F], F32, tag=tag, name=tag)

                    # independent chain (r1 -> b -> bx1 -> m2) runs on the
                    # otherwise-idle GPSIMD engine, concurrent with the DVE
                    # chain (r0 -> a -> t1/t2 -> s -> D0/m1)
                    r0 = T("r0")
                    nc.vector.tensor_scalar(r0[:], d30a[:], 0.0, DAMP, MAX, ADD)
                    r1 = T("r1")
                    nc.gpsimd.tensor_scalar(r1[:], d31a[:], 0.0, DAMP, MAX, ADD)
                    a_ = T("a")
                    nc.vector.tensor_mul(a_[:], r0[:], x0)
                    b_ = T("b")
                    nc.gpsimd.tensor_mul(b_[:], r1[:], x1)
                    t1 = T("t1")
                    nc.vector.tensor_mul(t1[:], a_[:], x0)
                    t2 = T("t2")
                    nc.vector.tensor_mul(t2[:], o3a[:], x1)
                    s_ = T("s")
                    nc.vector.tensor_add(s_[:], t1[:], t2[:])

                    D01 = out_pool.tile([spb, 2 * F], F32, tag="D01")
                    dv = D01[:].rearrange("p (f d) -> p f d", d=2)
                    D0v, D1v = dv[:, :, 0], dv[:, :, 1]
                    nc.vector.tensor_mul(D0v, a_[:], s_[:])

                    # D1 = c*s + b*(b*x1)  (c*s = a*c*x0 + c^2*x1)
                    bx1 = T("bx1")
                    nc.gpsimd.tensor_mul(bx1[:], b_[:], x1)
                    m1 = T("m1")
                    nc.vector.tensor_mul(m1[:], o3a[:], s_[:])
                    m2 = T("m2")
                    nc.gpsimd.tensor_mul(m2[:], b_[:], bx1[:])
                    nc.vector.tensor_add(D1v, m1[:], m2[:])

                    nc.sync.dma_start(y01v[blk], D01[:])

    nc.compile()
    return nc


def pack_weights(inputs):
    """Host-side packing of the tiny MLP weights into block-diag layout."""
    g = lambda k: np.asarray(inputs[k], dtype=np.float32)
    w_d1, w_d2, w_d3 = g("w_d1"), g("w_d2"), g("w_d3")
    w_o1, w_o2, w_o3 = g("w_o1"), g("w_o2"), g("w_o3")
    b_d1, b_d2, b_d3 = g("b_d1"), g("b_d2"), g("b_d3")
    b_o1, b_o2, b_o3 = g("b_o1"), g("b_o2"), g("b_o3")

    W1 = np.concatenate([w_d1, w_o1], axis=1)  # [2, 64]
    W2 = np.zeros((64, 64), np.float32)
    W2[:32, :32] = w_d2
    W2[32:, 32:] = w_o2
    W3 = np.zeros((64, 3), np.float32)
    W3[:32, 0:2] = w_d3
    W3[32:, 2:3] = w_o3

    W1p = np.zeros((4, 128), np.float32)
    W1p[0:2, 0:64] = W1
    W1p[2:4, 64:128] = W1
    W2p = np.zeros((128, 128), np.float32)
    W2p[:64, :64] = W2
    W2p[64:, 64:] = W2
    W3p = np.zeros((128, 32), np.float32)
    W3p[:64, 0:3] = W3
    W3p[64:, 3:6] = W3

    B1 = np.concatenate([b_d1, b_o1])  # [64]
    B1p = np.tile(B1, 2)[:, None].astype(np.float32)
    B2 = np.concatenate([b_d2, b_o2])
    B2p = np.tile(B2, 2)[:, None].astype(np.float32)

    bc3 = np.zeros((128, 1), np.float32)
    vals = [b_d3[0], b_d3[1], b_o3[0]]
    for r in range(128):
        if r % 32 < 6:
            bc3[r, 0] = vals[(r % 32) % 3]

    ball = np.concatenate([B1p, B2p, bc3], axis=1).astype(np.float32)
    return {
        "w1p": W1p.astype(np.float16),
        "w2p": W2p.astype(np.float16),
        "w3p": W3p.astype(np.float16),
        "ball": np.ascontiguousarray(ball),
    }


_CACHE = {}


def _get_program(bc, spb):
    key = (bc, spb)
    if key not in _CACHE:
        _CACHE[key] = build_program(bc, spb)
    return _CACHE[key]


LAST_RESULTS = None


def run(inputs, trace=False, n_cores=N_CORES):
    global LAST_RESULTS
    x = np.ascontiguousarray(np.asarray(inputs["x"], dtype=np.float32))
    B = x.shape[0]
    bc = B // n_cores
    packed = pack_weights(inputs)
    nc = _get_program(bc, 128 if bc % (128 * F) == 0 else 8)

    in_maps = []
    for i in range(n_cores):
        xs = np.ascontiguousarray(x[i * bc : (i + 1) * bc])
        # XTP[2t+d, c*F+f] = xs[c*2F + t*F + f, d]
        v = xs.reshape(bc // 1024, 2, 512, 2)
        xtp = np.ascontiguousarray(
            v.transpose(1, 3, 0, 2).reshape(4, bc // 2).astype(np.float16)
        )
        m = {"x": xs, "xt2": xtp}
        m.update(packed)
        in_maps.append(m)

    res = run_bass_kernel_spmd(
        nc, in_maps, core_ids=list(range(n_cores)), trace=trace
    )
    LAST_RESULTS = res
    y = np.concatenate([res.results[i]["y"] for i in range(n_cores)], axis=0)
    return y


def kernel(**inputs) -> np.ndarray:
    return run(inputs, trace=False)
